# revision 1
# baseline (speedup 1.0000x reference)
"""Trainium2 Bass kernel for nn_Attention_82540681494971.

Spatial self-attention block (LDM AttnBlock style, unscaled):
  qkv = conv1x1(x);  s = q^T k  [n x n] per (b,head);  attn = softmax(s, axis=-1)
  out[d,m] = sum_n v[d,n] attn[n,m];  y = conv1x1(out)

Shapes: B=4, C=64, H=W=64 -> n=4096 tokens, HEAD=4, d=16.

Sharding: 8 cores, core c handles batch b=c//2 and heads (0,1) if c%2==0
else (2,3). Each core computes a partial projection output over its two
heads' channels; host sums the two partials per batch and adds proj bias.

Key algebra: attn[n,m] = E[n,m]/rowsum[n] with E=exp(s). Since the AV
contraction runs over n (the softmax row index), fold 1/rowsum into v:
  out[d,m] = sum_n (v[d,n]*rinv[n]) E[n,m]
so the big E matrix never needs normalizing. Scores are exact-fp32-grade
via a 3-term bf16 split (q=q_hi+q_lo, k=k_hi+k_lo, drop lo*lo):
  s = [q_hi;q_lo;q_hi]^T [k_hi;k_hi;k_lo]   (K=48 stacked, 1 cyc/row)
"""

import numpy as np
from contextlib import ExitStack

import concourse.bass as bass
import concourse.mybir as mybir
import concourse.tile as tile
from concourse import bacc
from concourse.bass import ts, ds
from concourse.bass_utils import run_bass_kernel_spmd

F32 = mybir.dt.float32
BF16 = mybir.dt.bfloat16
AF = mybir.ActivationFunctionType

B, C, HEAD, D = 4, 64, 4, 16
N = 4096          # tokens = H*W
NT = 128          # n-tile (partition) size
NTILES = N // NT  # 32
MC = 512          # matmul free-dim chunk
SCH = (1536, 1536, 1024)  # scores/exp PSUM chunking

E_DT = BF16       # dtype of exp(s) and v~ for the AV matmul
G = 4             # max n-tiles per AV accumulation group
GROUPS = [4] * 7 + [2, 2]   # per-head group sizes (short tail groups)
ACCUM_LAST = False  # ACT accumulator does NOT persist across ACTIVATEs
SCORES_2GRP = True  # scores via two concurrent PE row-groups (K=48 each)
AV_COLTILE = False  # AV chains via concurrent PE col-groups (M=16 each)


def _body(tc, y, x1, wq, wk, wv, wp0, wp1):
    nc = tc.nc
    ctx = ExitStack()
    with ctx:
        pp = ctx.enter_context(tc.tile_pool(name="persist", bufs=1))
        cp = ctx.enter_context(tc.tile_pool(name="consts", bufs=1))

        # ---- constants ----
        wq_t = cp.tile([C + 1, 2 * D], F32)
        wk_t = cp.tile([C + 1, 2 * D], F32)
        wv_t = cp.tile([C + 1, 2 * D], F32)
        wp0_t = cp.tile([D, C], F32)
        wp1_t = cp.tile([D, C], F32)
        nc.sync.dma_start(wq_t[:], wq[:])
        nc.sync.dma_start(wk_t[:], wk[:])
        nc.sync.dma_start(wv_t[:], wv[:])
        nc.sync.dma_start(wp0_t[:], wp0[:])
        nc.sync.dma_start(wp1_t[:], wp1[:])

        # ---- persistent SBUF ----
        # [q_hi; q_lo; q_hi] stacked on partitions 0-47 and mirrored on
        # 64-111 (second PE row-group) when SCORES_2GRP. head-major cols.
        QKP = 64 + 3 * D if SCORES_2GRP else 3 * D
        qsp = pp.tile([QKP, 2 * N], BF16)
        ksp = pp.tile([QKP, 2 * N], BF16)
        vT_sb = pp.tile([NT, NTILES * 2 * D], F32)  # per n-tile: [128, 32] (h0|h1)
        out_h = [pp.tile([D, N], F32, tag=f"out{h}", name=f"out{h}")
                 for h in range(2)]
        y_sb = pp.tile([C, N], F32)

        # ---- phase 0: qkv + bf16 hi/lo split ----
        with (
            tc.tile_pool(name="x1p", bufs=1) as xp,
            tc.tile_pool(name="qkf", bufs=1) as qf,
            tc.tile_pool(name="spl", bufs=2) as spl,
            tc.tile_pool(name="p0psum", bufs=3, space="PSUM") as p0,
        ):
            x1_t = xp.tile([C + 1, N], F32)
            for i in range(8):  # parallel DMA queues
                nc.sync.dma_start(x1_t[:, ts(i, N // 8)], x1[:, ts(i, N // 8)])

            for which, w_t, dst, dup in (
                ("q", wq_t, qsp, 2),   # dup row-block 2 gets hi
                ("k", wk_t, ksp, 1),   # dup row-block 1 gets hi
            ):
                f32_sb = qf.tile([D, 2 * N], F32, tag="qkf32")
                for h in range(2):
                    for mc in range(N // MC):
                        ps = p0.tile([D, MC], F32, tag="p0")
                        nc.tensor.matmul(
                            ps[:], w_t[:, ts(h, D)], x1_t[:, ts(mc, MC)],
                            start=True, stop=True)
                        # alternate evacuation engine to halve critical path
                        dst_ap = f32_sb[:, ds(h * N + mc * MC, MC)]
                        if mc % 2 == 0:
                            nc.vector.tensor_copy(dst_ap, ps[:])
                        else:
                            nc.scalar.copy(dst_ap, ps[:])
                hi_t = spl.tile([D, 2 * N], BF16, tag="hi")
                lo_t = spl.tile([D, 2 * N], BF16, tag="lo")
                # hi-cast on ACT (idle in phase 0), lo-sub on DVE; halves
                nc.scalar.copy(hi_t[:, 0:N], f32_sb[:, 0:N])
                nc.scalar.copy(hi_t[:, N:2 * N], f32_sb[:, N:2 * N])
                nc.vector.tensor_sub(lo_t[:, 0:N], f32_sb[:, 0:N],
                                     hi_t[:, 0:N])
                nc.vector.tensor_sub(lo_t[:, N:2 * N], f32_sb[:, N:2 * N],
                                     hi_t[:, N:2 * N])
                # assemble K=48 stack via SBUF->SBUF DMA (no partition
                # alignment limits on DMA); chunked for queue parallelism
                lo_block = 1 if which == "q" else 2
                bases = [0, 64] if SCORES_2GRP else [0]
                for i in range(4):
                    sl = ts(i, N // 2)
                    for b0 in bases:
                        nc.sync.dma_start(dst[ds(b0, D), sl], hi_t[:, sl])
                        nc.sync.dma_start(
                            dst[ds(b0 + lo_block * D, D), sl], lo_t[:, sl])
                        nc.sync.dma_start(
                            dst[ds(b0 + dup * D, D), sl], hi_t[:, sl])

            for nt in range(NTILES):
                psv = p0.tile([NT, 2 * D], F32, tag="p0")
                nc.tensor.matmul(
                    psv[:], x1_t[:, ts(nt, NT)], wv_t[:],
                    start=True, stop=True)
                if nt % 2 == 0:
                    nc.vector.tensor_copy(vT_sb[:, ts(nt, 2 * D)], psv[:])
                else:
                    nc.scalar.copy(vT_sb[:, ts(nt, 2 * D)], psv[:])

        # ---- phase 1: attention, software-pipelined ----
        # Per step (one n-tile): emit this n-tile's score matmuls + exp,
        # woven with the AV chains of the *previous* group, so the PE queue
        # alternates score fills with AV matmuls. Keeps the PE busy (and
        # HAM-warm) while ACT streams exp, and keeps ACT fed during AV.
        MCN = N // MC

        def av_chain(ph, pg, pv, pe, mc):
            gl = len(pv)
            if AV_COLTILE:
                strip = 32 * (mc % 4)
                av = ap.tile([96 + D, MC], F32, tag="av",
                             name=f"av{ph}_{pg}_{mc}")
                out_ap = av[ds(strip, D), :]
                tp = (0, strip)
            else:
                av = ap.tile([D, MC], F32, tag="av", name=f"av{ph}_{pg}_{mc}")
                out_ap = av[:]
                tp = None
            for j in range(gl):
                nc.tensor.matmul(
                    out_ap, pv[j][:], pe[j][:, ts(mc, MC)],
                    start=(j == 0), stop=(j == gl - 1), tile_position=tp)
            dst = out_h[ph][:, ts(mc, MC)]
            if pg == 0:
                nc.vector.tensor_copy(dst, out_ap)
            else:
                nc.vector.tensor_add(dst, dst, out_ap)

        with (
            tc.tile_pool(name="ep", bufs=2 * G + 2) as ep,
            tc.tile_pool(name="rp", bufs=4) as rp,
            tc.tile_pool(name="vp", bufs=2 * G + 2) as vp,
            tc.tile_pool(name="sapsum", bufs=2, space="PSUM") as sp,
            tc.tile_pool(name="avpsum", bufs=2, space="PSUM") as ap,
        ):
            if ACCUM_LAST:
                # drain/clear the ACT accumulator of any phase-0 activity
                scratch = pp.tile([NT, 1], F32, name="acc_scratch")
                dump = pp.tile([NT, 1], F32, name="acc_dump")
                dump2 = pp.tile([NT, 1], F32, name="acc_dump2")
                nc.gpsimd.memset(scratch[:], 0.0)
                nc.scalar.activation(dump[:], scratch[:], AF.Identity,
                                     accum_out=dump2[:])

            prev = None  # (head, group, vts_tiles, e_tiles)
            for h in range(2):
                nt0 = 0
                for g, gsz in enumerate(GROUPS):
                    e_tiles, vts_tiles = [], []
                    for j in range(gsz):
                        nt = nt0 + j
                        if prev is not None:
                            c0 = MCN * j // gsz
                            c1 = MCN * (j + 1) // gsz
                            chains = list(range(c0, c1))
                        else:
                            chains = []
                        e_t = ep.tile([NT, N], E_DT, tag="e",
                                      name=f"e{h}_{nt}")
                        rsp = rp.tile([NT, 4], F32, tag="rs", name="rsp")
                        off = 0
                        for ci, csz in enumerate(SCH):
                            s_ps = sp.tile([NT, SCH[0]], F32, tag="sa",
                                           name="s_ps")
                            for i in range(csz // MC):
                                if SCORES_2GRP and i % 2 == 1:
                                    b0, tp = 64, (64, 0)
                                else:
                                    b0, tp = 0, (0, 0) if SCORES_2GRP else None
                                nc.tensor.matmul(
                                    s_ps[:, ts(i, MC)],
                                    qsp[ds(b0, 3 * D),
                                        ds(h * N + nt * NT, NT)],
                                    ksp[ds(b0, 3 * D),
                                        ds(h * N + off + i * MC, MC)],
                                    start=True, stop=True, tile_position=tp)
                            want_acc = (not ACCUM_LAST) or ci == len(SCH) - 1
                            if want_acc:
                                acc = rsp[:, 0:1] if ACCUM_LAST \
                                    else rsp[:, ds(ci, 1)]
                                nc.scalar.activation(
                                    e_t[:, ds(off, csz)], s_ps[:, :csz],
                                    AF.Exp, accum_out=acc)
                            else:
                                nc.scalar.activation(
                                    e_t[:, ds(off, csz)], s_ps[:, :csz],
                                    AF.Exp)
                            off += csz
                            # weave one prev-group AV chain between chunks
                            if chains:
                                ph_, pg_, pv_, pe_ = prev
                                av_chain(ph_, pg_, pv_, pe_, chains.pop(0))
                        if prev is not None and chains:
                            ph_, pg_, pv_, pe_ = prev
                            for mc in chains:
                                av_chain(ph_, pg_, pv_, pe_, mc)
                        rinv = rp.tile([NT, 1], F32, tag="ri", name="rinv")
                        if ACCUM_LAST:
                            nc.vector.reciprocal(rinv[:], rsp[:, 0:1])
                        else:
                            rs = rp.tile([NT, 1], F32, tag="r1", name="rs")
                            nc.vector.reduce_sum(
                                rs[:], rsp[:, 0:3], axis=mybir.AxisListType.X)
                            nc.vector.reciprocal(rinv[:], rs[:])
                        vts = vp.tile([NT, D], E_DT, tag="vts",
                                      name=f"vts{h}_{nt}")
                        nc.gpsimd.tensor_scalar_mul(
                            vts[:], vT_sb[:, ds(nt * 2 * D + h * D, D)],
                            rinv[:])
                        e_tiles.append(e_t)
                        vts_tiles.append(vts)
                    prev = (h, g, vts_tiles, e_tiles)
                    nt0 += gsz

            # ---- tail: flush last group's chains, then projection ----
            ph_, pg_, pv_, pe_ = prev
            for mc in range(MCN):
                av_chain(ph_, pg_, pv_, pe_, mc)
            for mc in range(MCN):
                yp = ap.tile([C, MC], F32, tag="av", name=f"yp{mc}")
                nc.tensor.matmul(yp[:], wp0_t[:], out_h[0][:, ts(mc, MC)],
                                 start=True, stop=False)
                nc.tensor.matmul(yp[:], wp1_t[:], out_h[1][:, ts(mc, MC)],
                                 start=False, stop=True)
                if mc % 2 == 0:
                    nc.vector.tensor_copy(y_sb[:, ts(mc, MC)], yp[:])
                else:
                    nc.scalar.copy(y_sb[:, ts(mc, MC)], yp[:])
                nc.sync.dma_start(y[:, ts(mc, MC)], y_sb[:, ts(mc, MC)])


_PROGRAM = None


def _get_program():
    global _PROGRAM
    if _PROGRAM is None:
        nc = bacc.Bacc("TRN2", target_bir_lowering=False, debug=False,
                       num_devices=8)
        x1 = nc.dram_tensor("x1", [C + 1, N], F32, kind="ExternalInput").ap()
        wq = nc.dram_tensor("wq", [C + 1, 2 * D], F32, kind="ExternalInput").ap()
        wk = nc.dram_tensor("wk", [C + 1, 2 * D], F32, kind="ExternalInput").ap()
        wv = nc.dram_tensor("wv", [C + 1, 2 * D], F32, kind="ExternalInput").ap()
        wp0 = nc.dram_tensor("wp0", [D, C], F32, kind="ExternalInput").ap()
        wp1 = nc.dram_tensor("wp1", [D, C], F32, kind="ExternalInput").ap()
        y = nc.dram_tensor("y", [C, N], F32, kind="ExternalOutput").ap()
        with tile.TileContext(nc) as tc:
            _body(tc, y, x1, wq, wk, wv, wp0, wp1)
        nc.compile()
        _PROGRAM = nc
    return _PROGRAM


def _make_in_maps(x, qkv_w, qkv_b, proj_w, proj_b=None):
    x = np.asarray(x, dtype=np.float32)
    qkv_w = np.asarray(qkv_w, dtype=np.float32)
    qkv_b = np.asarray(qkv_b, dtype=np.float32)
    proj_w = np.asarray(proj_w, dtype=np.float32)

    in_maps = []
    for core in range(8):
        b = core // 2
        h0 = 2 * (core % 2)
        heads = (h0, h0 + 1)
        x1 = np.concatenate(
            [x[b].reshape(C, N), np.ones((1, N), np.float32)], axis=0)

        def aug_qk(block):
            w = np.empty((C + 1, 2 * D), np.float32)
            for j, h in enumerate(heads):
                rows = slice(block * C + h * D, block * C + (h + 1) * D)
                w[:C, j * D:(j + 1) * D] = qkv_w[rows, :].T
                w[C, j * D:(j + 1) * D] = qkv_b[rows]
            return w

        wp_parts = [
            np.ascontiguousarray(proj_w[:, h * D:(h + 1) * D].T)
            for h in heads
        ]

        in_maps.append({
            "x1": np.ascontiguousarray(x1),
            "wq": aug_qk(0),
            "wk": aug_qk(1),
            "wv": aug_qk(2),
            "wp0": wp_parts[0],
            "wp1": wp_parts[1],
        })
    return in_maps


def run_cores(inputs, **kw):
    """Compile+run on the 8 cores; returns BassKernelResults."""
    nc = _get_program()
    in_maps = _make_in_maps(**inputs)
    return run_bass_kernel_spmd(nc, in_maps, list(range(8)), **kw)


def kernel(x, qkv_w, qkv_b, proj_w, proj_b):
    res = run_cores(dict(x=x, qkv_w=qkv_w, qkv_b=qkv_b,
                         proj_w=proj_w, proj_b=proj_b))
    proj_b = np.asarray(proj_b, dtype=np.float32)
    parts = [r["y"] for r in res.results]
    out = np.empty((B, C, N), np.float32)
    for b in range(B):
        out[b] = parts[2 * b] + parts[2 * b + 1] + proj_b[:, None]
    return out.reshape(B, C, 64, 64)


if __name__ == "__main__":
    _get_program()
    print("program built OK")



# revision 2
# speedup vs baseline: 1.3885x; 1.3885x over previous
"""Trainium2 Bass kernel for nn_Attention_82540681494971.

Spatial self-attention block (LDM AttnBlock style, unscaled):
  qkv = conv1x1(x);  s = q^T k  [n x n] per (b,head);  attn = softmax(s, axis=-1)
  out[d,m] = sum_n v[d,n] attn[n,m];  y = conv1x1(out)

Shapes: B=4, C=64, H=W=64 -> n=4096 tokens, HEAD=4, d=16.

Sharding: 8 cores, core c handles batch b=c//2 and heads (0,1) if c%2==0
else (2,3). Each core computes a partial projection output over its two
heads' channels; host sums the two partials per batch and adds proj bias.

Key algebra: attn[n,m] = E[n,m]/rowsum[n] with E=exp(s). Since the AV
contraction runs over n (the softmax row index), fold 1/rowsum into v:
  out[d,m] = sum_n (v[d,n]*rinv[n]) E[n,m]
so the big E matrix never needs normalizing. Scores are exact-fp32-grade
via a 3-term bf16 split (q=q_hi+q_lo, k=k_hi+k_lo, drop lo*lo):
  s = [q_hi;q_lo;q_hi]^T [k_hi;k_hi;k_lo]   (K=48 stacked, 1 cyc/row)

v2 design (ACT-roofline targeted):
  - ACT does exp ONLY (no accum_out): rowsums via DVE reduce over bf16 E.
  - AV runs 4 concurrent PSUM-accumulation chains in PE col-groups
    (tile_position=(0,32s)); strip s owns m-chunks {s, 4+s}. The two
    [112,512] PSUM accumulators persist across all 8 groups of a head
    (~4x less PE time than chained [16,512] AV + no per-chain evac).
  - Projection col-tiled over 4 row-groups in bf16, reusing AV psum slots.
  - PSUM: 6 banks scores (2 x [128,1536] double-buffer) + 2 banks AV.
"""

import numpy as np
from contextlib import ExitStack

import concourse.bass as bass
import concourse.mybir as mybir
import concourse.tile as tile
from concourse import bacc
from concourse.bass import ts, ds
from concourse.bass_utils import run_bass_kernel_spmd

F32 = mybir.dt.float32
BF16 = mybir.dt.bfloat16
AF = mybir.ActivationFunctionType

B, C, HEAD, D = 4, 64, 4, 16
N = 4096          # tokens = H*W
NT = 128          # n-tile (partition) size
NTILES = N // NT  # 32
MC = 512          # matmul free-dim chunk
MCN = N // MC     # 8 m-chunks
SCH = (1536, 1536, 1024)  # scores/exp PSUM chunking (3+3 banks, 2 bufs)
G = 4             # n-tiles per AV group
NGROUPS = NTILES // G   # 8


def _body(tc, y, x1, wq, wk, wv, wp0, wp1):
    nc = tc.nc
    ctx = ExitStack()
    with ctx:
        pp = ctx.enter_context(tc.tile_pool(name="persist", bufs=1))
        cp = ctx.enter_context(tc.tile_pool(name="consts", bufs=1))

        # ---- warm the exp table while DMAs run ----
        zz = pp.tile([NT, 1], F32)
        zz2 = pp.tile([NT, 1], F32)
        nc.gpsimd.memset(zz[:], 0.0)
        nc.scalar.activation(zz2[:], zz[:], AF.Exp)

        # ---- constants ----
        wq_t = cp.tile([C + 1, 2 * D], F32)
        wk_t = cp.tile([C + 1, 2 * D], F32)
        wv_t = cp.tile([C + 1, 2 * D], F32)
        wp0_f = cp.tile([112, C], F32)
        wp1_f = cp.tile([112, C], F32)
        nc.sync.dma_start(wq_t[:], wq[:])
        nc.sync.dma_start(wk_t[:], wk[:])
        nc.sync.dma_start(wv_t[:], wv[:])
        nc.sync.dma_start(wp0_f[:], wp0[:])
        nc.sync.dma_start(wp1_f[:], wp1[:])
        wp_t = [cp.tile([112, C], BF16, tag="wpb", name=f"wpb{h}", bufs=2)
                for h in range(2)]
        nc.vector.tensor_copy(wp_t[0][:], wp0_f[:])
        nc.vector.tensor_copy(wp_t[1][:], wp1_f[:])

        # ---- persistent SBUF ----
        # [q_hi; q_lo; q_hi] stacked on partitions 0-47 and mirrored on
        # 64-111 (second PE row-group). head-major cols.
        qsp = pp.tile([112, 2 * N], BF16)
        ksp = pp.tile([112, 2 * N], BF16)
        vT_sb = pp.tile([NT, NTILES * 2 * D], F32)  # per n-tile: [128,32] (h0|h1)
        # AV outputs, strip-major: out_sb[h][32s:32s+16, t*512:] = chunk 4t+s
        out_sb = [pp.tile([112, 2 * MC], BF16, tag=f"osb{h}", name=f"osb{h}")
                  for h in range(2)]
        y_sb = pp.tile([C, N], F32)

        # ---- phase 0: qkv + bf16 hi/lo split ----
        with (
            tc.tile_pool(name="x1p", bufs=1) as xp,
            tc.tile_pool(name="qkf", bufs=1) as qf,
            tc.tile_pool(name="spl", bufs=2) as spl,
            tc.tile_pool(name="p0psum", bufs=3, space="PSUM") as p0,
        ):
            x1_t = xp.tile([C + 1, N], F32)
            for i in range(8):  # parallel DMA queues
                nc.sync.dma_start(x1_t[:, ts(i, N // 8)], x1[:, ts(i, N // 8)])

            for which, w_t, dst, dup in (
                ("q", wq_t, qsp, 2),   # dup row-block 2 gets hi
                ("k", wk_t, ksp, 1),   # dup row-block 1 gets hi
            ):
                f32_sb = qf.tile([D, 2 * N], F32, tag="qkf32")
                for h in range(2):
                    for mc in range(N // MC):
                        ps = p0.tile([D, MC], F32, tag="p0")
                        nc.tensor.matmul(
                            ps[:], w_t[:, ts(h, D)], x1_t[:, ts(mc, MC)],
                            start=True, stop=True)
                        # alternate evacuation engine to halve critical path
                        dst_ap = f32_sb[:, ds(h * N + mc * MC, MC)]
                        if mc % 2 == 0:
                            nc.vector.tensor_copy(dst_ap, ps[:])
                        else:
                            nc.scalar.copy(dst_ap, ps[:])
                hi_t = spl.tile([D, 2 * N], BF16, tag="hi")
                lo_t = spl.tile([D, 2 * N], BF16, tag="lo")
                # hi-cast on ACT (idle in phase 0), lo-sub on DVE; halves
                nc.scalar.copy(hi_t[:, 0:N], f32_sb[:, 0:N])
                nc.scalar.copy(hi_t[:, N:2 * N], f32_sb[:, N:2 * N])
                nc.vector.tensor_sub(lo_t[:, 0:N], f32_sb[:, 0:N],
                                     hi_t[:, 0:N])
                nc.vector.tensor_sub(lo_t[:, N:2 * N], f32_sb[:, N:2 * N],
                                     hi_t[:, N:2 * N])
                # assemble K=48 stack via SBUF->SBUF DMA (no partition
                # alignment limits on DMA); chunked for queue parallelism
                lo_block = 1 if which == "q" else 2
                for i in range(4):
                    sl = ts(i, N // 2)
                    for b0 in (0, 64):
                        nc.sync.dma_start(dst[ds(b0, D), sl], hi_t[:, sl])
                        nc.sync.dma_start(
                            dst[ds(b0 + lo_block * D, D), sl], lo_t[:, sl])
                        nc.sync.dma_start(
                            dst[ds(b0 + dup * D, D), sl], hi_t[:, sl])

            for nt in range(NTILES):
                psv = p0.tile([NT, 2 * D], F32, tag="p0")
                nc.tensor.matmul(
                    psv[:], x1_t[:, ts(nt, NT)], wv_t[:],
                    start=True, stop=True)
                if nt % 2 == 0:
                    nc.vector.tensor_copy(vT_sb[:, ts(nt, 2 * D)], psv[:])
                else:
                    nc.scalar.copy(vT_sb[:, ts(nt, 2 * D)], psv[:])

        # ---- phase 1: attention, software-pipelined ----
        # Per n-tile: 3 score-chunk matmul bursts feed 3 exp ACTIVATEs (ACT
        # exp-only). Rowsum on DVE from the bf16 E tile. AV rounds of the
        # *previous* group weave between score chunks: each round is 16
        # matmuls over 4 concurrent col-group strips accumulating into
        # persistent [112,512] PSUM accumulators (2 per head).
        with (
            tc.tile_pool(name="ep", bufs=2 * G + 2) as ep,
            tc.tile_pool(name="rp", bufs=4) as rp,
            tc.tile_pool(name="vp", bufs=2 * G + 2) as vp,
            tc.tile_pool(name="sapsum", bufs=2, space="PSUM") as sp,
            tc.tile_pool(name="avpsum", bufs=2, space="PSUM") as ap,
        ):
            pending = []   # queued closures: AV rounds + accum evacs

            def emit_pending():
                if pending:
                    pending.pop(0)()

            for h in range(2):
                av_ts = [ap.tile([112, MC], F32, tag="av", name=f"av{h}_{t}")
                         for t in range(2)]
                started = [[False] * 4 for _ in range(2)]

                def make_round(h_, av_ts_, started_, t, vls, els, is_last):
                    def go():
                        gl = len(vls)
                        for j in range(gl):
                            for s in range(4):
                                first = not started_[t][s]
                                started_[t][s] = True
                                nc.tensor.matmul(
                                    av_ts_[t][ds(32 * s, D), :],
                                    vls[j][:],
                                    els[j][:, ts(4 * t + s, MC)],
                                    start=first,
                                    stop=(is_last and j == gl - 1),
                                    tile_position=(0, 32 * s))
                    return go

                def make_evac(h_, av_ts_):
                    def go():
                        for t in range(2):
                            nc.vector.tensor_copy(
                                out_sb[h_][:, ts(t, MC)], av_ts_[t][:])
                    return go

                for g in range(NGROUPS):
                    e_tiles, vts_tiles = [], []
                    for j in range(G):
                        nt = g * G + j
                        e_t = ep.tile([NT, N], BF16, tag="e",
                                      name=f"e{h}_{nt}")
                        off = 0
                        for ci, csz in enumerate(SCH):
                            s_ps = sp.tile([NT, SCH[0]], F32, tag="sa",
                                           name="s_ps")
                            for i in range(csz // MC):
                                if i % 2 == 1:
                                    b0, tp = 64, (64, 0)
                                else:
                                    b0, tp = 0, (0, 0)
                                nc.tensor.matmul(
                                    s_ps[:, ts(i, MC)],
                                    qsp[ds(b0, 3 * D),
                                        ds(h * N + nt * NT, NT)],
                                    ksp[ds(b0, 3 * D),
                                        ds(h * N + off + i * MC, MC)],
                                    start=True, stop=True, tile_position=tp)
                            nc.scalar.activation(
                                e_t[:, ds(off, csz)], s_ps[:, :csz], AF.Exp)
                            off += csz
                            # weave queued AV work between chunks
                            if ci == 0 and (j % 2 == 1 or len(pending) > 2):
                                emit_pending()
                        # rowsum + 1/rowsum + fold into v (DVE + gpsimd)
                        rs = rp.tile([NT, 1], F32, tag="rs", name="rs")
                        nc.vector.reduce_sum(
                            rs[:], e_t[:], axis=mybir.AxisListType.X)
                        rinv = rp.tile([NT, 1], F32, tag="ri", name="rinv")
                        nc.vector.reciprocal(rinv[:], rs[:])
                        vts = vp.tile([NT, D], BF16, tag="vts",
                                      name=f"vts{h}_{nt}")
                        nc.gpsimd.tensor_scalar_mul(
                            vts[:], vT_sb[:, ds(nt * 2 * D + h * D, D)],
                            rinv[:])
                        e_tiles.append(e_t)
                        vts_tiles.append(vts)
                    is_last = g == NGROUPS - 1
                    for t in range(2):
                        pending.append(make_round(
                            h, av_ts, started, t, vts_tiles, e_tiles,
                            is_last))
                    if is_last:
                        pending.append(make_evac(h, av_ts))

            # ---- tail: flush remaining AV + evac, then projection ----
            while pending:
                emit_pending()
            for mc in range(MCN):
                s, t = mc % 4, mc // 4
                yp_t = ap.tile([112, MC], F32, tag="av", name=f"yp{mc}")
                yp = yp_t[ds(0, C), :]
                nc.tensor.matmul(
                    yp, wp_t[0][ds(32 * s, D), :],
                    out_sb[0][ds(32 * s, D), ts(t, MC)],
                    start=True, stop=False, tile_position=(32 * s, 0))
                nc.tensor.matmul(
                    yp, wp_t[1][ds(32 * s, D), :],
                    out_sb[1][ds(32 * s, D), ts(t, MC)],
                    start=False, stop=True, tile_position=(32 * s, 0))
                if mc % 2 == 0:
                    nc.vector.tensor_copy(y_sb[:, ts(mc, MC)], yp)
                else:
                    nc.scalar.copy(y_sb[:, ts(mc, MC)], yp)
                nc.sync.dma_start(y[:, ts(mc, MC)], y_sb[:, ts(mc, MC)])


_PROGRAM = None


def _get_program():
    global _PROGRAM
    if _PROGRAM is None:
        nc = bacc.Bacc("TRN2", target_bir_lowering=False, debug=False,
                       num_devices=8)
        x1 = nc.dram_tensor("x1", [C + 1, N], F32, kind="ExternalInput").ap()
        wq = nc.dram_tensor("wq", [C + 1, 2 * D], F32, kind="ExternalInput").ap()
        wk = nc.dram_tensor("wk", [C + 1, 2 * D], F32, kind="ExternalInput").ap()
        wv = nc.dram_tensor("wv", [C + 1, 2 * D], F32, kind="ExternalInput").ap()
        wp0 = nc.dram_tensor("wp0", [112, C], F32, kind="ExternalInput").ap()
        wp1 = nc.dram_tensor("wp1", [112, C], F32, kind="ExternalInput").ap()
        y = nc.dram_tensor("y", [C, N], F32, kind="ExternalOutput").ap()
        with tile.TileContext(nc) as tc:
            _body(tc, y, x1, wq, wk, wv, wp0, wp1)
        nc.compile()
        _PROGRAM = nc
    return _PROGRAM


def _make_in_maps(x, qkv_w, qkv_b, proj_w, proj_b=None):
    x = np.asarray(x, dtype=np.float32)
    qkv_w = np.asarray(qkv_w, dtype=np.float32)
    qkv_b = np.asarray(qkv_b, dtype=np.float32)
    proj_w = np.asarray(proj_w, dtype=np.float32)

    in_maps = []
    for core in range(8):
        b = core // 2
        h0 = 2 * (core % 2)
        heads = (h0, h0 + 1)
        x1 = np.concatenate(
            [x[b].reshape(C, N), np.ones((1, N), np.float32)], axis=0)

        def aug_qk(block):
            w = np.empty((C + 1, 2 * D), np.float32)
            for j, h in enumerate(heads):
                rows = slice(block * C + h * D, block * C + (h + 1) * D)
                w[:C, j * D:(j + 1) * D] = qkv_w[rows, :].T
                w[C, j * D:(j + 1) * D] = qkv_b[rows]
            return w

        def wp_rep(h):
            w = np.zeros((112, C), np.float32)
            blk = proj_w[:, h * D:(h + 1) * D].T  # [D, C]
            for s in range(4):
                w[32 * s:32 * s + D, :] = blk
            return w

        in_maps.append({
            "x1": np.ascontiguousarray(x1),
            "wq": aug_qk(0),
            "wk": aug_qk(1),
            "wv": aug_qk(2),
            "wp0": wp_rep(heads[0]),
            "wp1": wp_rep(heads[1]),
        })
    return in_maps


def run_cores(inputs, **kw):
    """Compile+run on the 8 cores; returns BassKernelResults."""
    nc = _get_program()
    in_maps = _make_in_maps(**inputs)
    return run_bass_kernel_spmd(nc, in_maps, list(range(8)), **kw)


def kernel(x, qkv_w, qkv_b, proj_w, proj_b):
    res = run_cores(dict(x=x, qkv_w=qkv_w, qkv_b=qkv_b,
                         proj_w=proj_w, proj_b=proj_b))
    proj_b = np.asarray(proj_b, dtype=np.float32)
    parts = [r["y"] for r in res.results]
    out = np.empty((B, C, N), np.float32)
    for b in range(B):
        out[b] = parts[2 * b] + parts[2 * b + 1] + proj_b[:, None]
    return out.reshape(B, C, 64, 64)


if __name__ == "__main__":
    _get_program()
    print("program built OK")
